# revision 17
# baseline (speedup 1.0000x reference)
"""Trainium2 Bass kernel for DeformationTrackerBiFlowModel (non-teacher-forcing).

Reference math (per batch element b, per step t):
    x_t   = [prev_out (2), fin_t (3)]            (5,)
    h_t   = tanh(x_t @ W_rnn + b_rnn)            (12,)   (U_rnn is inert: h0 == 0)
    out_t = [cp0 (2), h_t (12)] @ W_out + b_out  (2,)
    prev_out_{t+1} = out_t;  prev_out_0 = cp0

The autoregressive chain limits wall time to T * (per-step dependency
latency), so the recurrence is algebraically refactored to keep only TWO ops
on the chain (matmul -> tanh):

    pre_{t+1} = h_t @ (Wo2 @ W1p)                          [mmH, ON chain]
              + [fin_{t+1}, 1, cp0] @ wF                   [mmF, off chain]
    h_{t+1}   = tanh(pre_{t+1})                            [ACT, ON chain]
    out_t     = h_t @ Wo2 |psum| + (cp0 @ Wo1 + b_out)     [mmO + DVE add,
                                                            off chain]
  where W1p = W_rnn[:2], Wo1 = W_out[:2], Wo2 = W_out[2:], and wF packs
  W_rnn[2:], a ones-row carrying (b_rnn + b_out @ W1p), and cp0-rows carrying
  (Wo1 @ W1p). Step 0 uses wF0 (direct W1p / b_rnn rows) with no mmH.

Device mapping: batch 65536 sharded over 8 cores (8192 each, padded to
8200 = G*C*COLS); features on SBUF partitions with G=10 trajectories packed
block-diagonally per matmul; C=2 independent column chains of COLS=410.
"""

import os
from contextlib import ExitStack

import numpy as np

import concourse.mybir as mybir
import concourse.tile as tile
from concourse import bacc
from concourse.bass_utils import run_bass_kernel_spmd

B, T = 65536, 100
D_CP, D_FIN, HID = 2, 3, 12
NCORES = 8
BC = B // NCORES              # 8192 per core
G = 10                        # trajectories packed per matmul (block-diag)
C = 2                         # independent column chains
COLS = 410                    # batch columns per chain
BP = G * C * COLS             # 8200 padded batch per core
XROWS = D_FIN * G + 1 + D_CP * G   # fin rows + ones row + cp0 rows = 51

F32 = mybir.dt.float32

# Matmul-path dtype. bf16 streams 1 col/cycle on the PE; float32r measured
# ~1.6-2 cycles/col; exact fp32 is 4 cycles/col.
# Select with DTB_MM in {bf16, f32r, f32}; default bf16.
_MM_CHOICES = {"bf16": mybir.dt.bfloat16, "f32r": mybir.dt.float32r, "f32": F32}
MM_DTYPE = _MM_CHOICES[os.environ.get("DTB_MM", "bf16")]
MM_NP = mybir.dt.np(MM_DTYPE)  # numpy dtype of device matmul-path tensors

LAST_RESULTS = None  # test.py introspects profiling info from here


def build_program(t_steps=T, g=G, c=C, cols=COLS, mm_dtype=None):
    if mm_dtype is None:
        mm_dtype = MM_DTYPE
    XDT = mm_dtype
    xrows = D_FIN * g + 1 + D_CP * g
    nc = bacc.Bacc(target_bir_lowering=False)

    fin = nc.dram_tensor("fin", [t_steps, c, xrows, cols], XDT, kind="ExternalInput")
    cb = nc.dram_tensor("cb", [c, D_CP * g, cols], F32, kind="ExternalInput")
    wf = nc.dram_tensor("wf", [xrows, HID * g], XDT, kind="ExternalInput")
    wf0 = nc.dram_tensor("wf0", [xrows, HID * g], XDT, kind="ExternalInput")
    wh = nc.dram_tensor("wh", [HID * g, HID * g], XDT, kind="ExternalInput")
    wo = nc.dram_tensor("wo", [HID * g, D_CP * g], XDT, kind="ExternalInput")
    out = nc.dram_tensor("out", [t_steps, c, D_CP * g, cols], XDT, kind="ExternalOutput")

    tanh = mybir.ActivationFunctionType.Tanh

    with tile.TileContext(nc) as tc, ExitStack() as ctx:
        const = ctx.enter_context(tc.tile_pool(name="const", bufs=1))
        xpool = ctx.enter_context(tc.tile_pool(name="xpool", bufs=4))
        hpool = ctx.enter_context(tc.tile_pool(name="hpool", bufs=3))
        opool = ctx.enter_context(tc.tile_pool(name="opool", bufs=3))
        psum = ctx.enter_context(tc.tile_pool(name="psum", bufs=2, space="PSUM"))

        wfs = const.tile([xrows, HID * g], XDT, name="wfs")
        nc.sync.dma_start(out=wfs, in_=wf[:, :])
        wf0s = const.tile([xrows, HID * g], XDT, name="wf0s")
        nc.sync.dma_start(out=wf0s, in_=wf0[:, :])
        whs = const.tile([HID * g, HID * g], XDT, name="whs")
        nc.sync.dma_start(out=whs, in_=wh[:, :])
        wos = const.tile([HID * g, D_CP * g], XDT, name="wos")
        nc.sync.dma_start(out=wos, in_=wo[:, :])
        cbs = []
        for ch in range(c):
            cbt = const.tile([D_CP * g, cols], F32, tag=f"cb{ch}", name=f"cbs{ch}")
            nc.sync.dma_start(out=cbt, in_=cb[ch])
            cbs.append(cbt)

        xts = []
        for ch in range(c):
            xt = xpool.tile([xrows, cols], XDT, tag=f"x{ch}", name=f"x_{ch}_0")
            nc.sync.dma_start(out=xt, in_=fin[0, ch])
            xts.append(xt)

        hs = [None] * c
        for t in range(t_steps):
            for ch in range(c):
                p1 = psum.tile([HID * g, cols], F32, tag=f"p1{ch}", name=f"p1_{ch}_{t}")
                if t == 0:
                    nc.tensor.matmul(p1, wf0s, xts[ch], start=True, stop=True)
                else:
                    # mmF first (inputs ready ahead of time); mmH closes the
                    # accumulation group and is the only cross-step dependency.
                    nc.tensor.matmul(p1, wfs, xts[ch], start=True, stop=False)
                    nc.tensor.matmul(p1, whs, hs[ch], start=False, stop=True)
                h = hpool.tile([HID * g, cols], XDT, tag=f"h{ch}", name=f"h_{ch}_{t}")
                nc.scalar.activation(h, p1, tanh)
                hs[ch] = h

                p2 = psum.tile([D_CP * g, cols], F32, tag=f"p2{ch}", name=f"p2_{ch}_{t}")
                nc.tensor.matmul(p2, wos, h, start=True, stop=True)
                osb = opool.tile([D_CP * g, cols], XDT, tag=f"o{ch}", name=f"o_{ch}_{t}")
                nc.vector.tensor_add(osb, p2, cbs[ch])
                nc.sync.dma_start(out=out[t, ch], in_=osb)

                if t + 1 < t_steps:
                    xn = xpool.tile([xrows, cols], XDT, tag=f"x{ch}", name=f"x_{ch}_{t + 1}")
                    nc.sync.dma_start(out=xn, in_=fin[t + 1, ch])
                    xts[ch] = xn
    nc.compile()
    return nc


def build_packed_weights(W_rnn, W_out, b_rnn, b_out, g=G):
    W_rnn = np.asarray(W_rnn, np.float32)
    W_out = np.asarray(W_out, np.float32)
    b_rnn = np.asarray(b_rnn, np.float32)
    b_out = np.asarray(b_out, np.float32)
    W1p, W1f = W_rnn[:D_CP], W_rnn[D_CP:]
    Wo1, Wo2 = W_out[:D_CP], W_out[D_CP:]
    xrows = D_FIN * g + 1 + D_CP * g
    ones_row = D_FIN * g

    wf = np.zeros((xrows, HID * g), np.float32)
    wf0 = np.zeros((xrows, HID * g), np.float32)
    wh = np.zeros((HID * g, HID * g), np.float32)
    wo = np.zeros((HID * g, D_CP * g), np.float32)
    E = Wo1 @ W1p                      # (2, 12) cp0 contribution to next pre
    r = b_rnn + b_out @ W1p            # (12,) ones-row weight (steady state)
    Wh = Wo2 @ W1p                     # (12, 12) h contribution to next pre
    for i in range(g):
        hsl = slice(HID * i, HID * (i + 1))
        wf[D_FIN * i : D_FIN * (i + 1), hsl] = W1f
        wf0[D_FIN * i : D_FIN * (i + 1), hsl] = W1f
        wf[ones_row, hsl] = r
        wf0[ones_row, hsl] = b_rnn
        csl = slice(ones_row + 1 + D_CP * i, ones_row + 1 + D_CP * (i + 1))
        wf[csl, hsl] = E
        wf0[csl, hsl] = W1p
        wh[hsl, hsl] = Wh
        wo[hsl, D_CP * i : D_CP * (i + 1)] = Wo2
    return wf, wf0, wh, wo


def stage_inputs(cp0, fin, cvec, g=G, c=C, cols=COLS, t_steps=T):
    """Per-core staging: batch-major -> feature-major device layouts.

    fin_d rows per (t, chain): [fin (3G) | ones (1) | cp0 (2G)].
    """
    bp = g * c * cols
    bc = cp0.shape[0]
    xrows = D_FIN * g + 1 + D_CP * g
    fin_p = np.zeros((bp, t_steps, D_FIN), np.float32)
    fin_p[:bc] = fin
    cp0_p = np.zeros((bp, D_CP), np.float32)
    cp0_p[:bc] = cp0
    cv_p = np.zeros((bp, D_CP), np.float32)
    cv_p[:bc] = cvec
    # b = ch*(g*cols) + gi*cols + j
    fin_d = np.ones((t_steps, c, xrows, cols), np.float32)
    fin_d[:, :, : D_FIN * g, :] = fin_p.reshape(c, g, cols, t_steps, D_FIN).transpose(
        3, 0, 1, 4, 2
    ).reshape(t_steps, c, D_FIN * g, cols)
    cp0_d = cp0_p.reshape(c, g, cols, D_CP).transpose(0, 1, 3, 2).reshape(
        c, D_CP * g, cols
    )
    fin_d[:, :, D_FIN * g + 1 :, :] = cp0_d[None]
    cb_d = np.ascontiguousarray(
        cv_p.reshape(c, g, cols, D_CP).transpose(0, 1, 3, 2)
    ).reshape(c, D_CP * g, cols)
    return fin_d, cb_d


def unstage_output(out_d, bc, g=G, c=C, cols=COLS, t_steps=T):
    """(T, C, 2G, COLS) device layout -> (bc, T, 2) batch-major."""
    bp = g * c * cols
    o = out_d.reshape(t_steps, c, g, D_CP, cols).transpose(1, 2, 4, 0, 3)
    return np.ascontiguousarray(o).reshape(bp, t_steps, D_CP)[:bc]


def kernel(control_point_input, finger_input, W_rnn, U_rnn, b_rnn, W_out, b_out):
    global LAST_RESULTS
    cp = np.asarray(control_point_input, np.float32)
    fin = np.asarray(finger_input, np.float32)
    W_rnn = np.asarray(W_rnn, np.float32)
    b_rnn = np.asarray(b_rnn, np.float32)
    W_out = np.asarray(W_out, np.float32)
    b_out = np.asarray(b_out, np.float32)

    cp0 = cp[:, 0, :]                                  # (B, 2)
    cvec = cp0 @ W_out[:D_CP] + b_out                  # (B, 2), constant per step
    wf, wf0, wh, wo = build_packed_weights(W_rnn, W_out, b_rnn, b_out)
    wf, wf0, wh, wo = (x.astype(MM_NP) for x in (wf, wf0, wh, wo))

    nc = build_program()
    in_maps = []
    for m in range(NCORES):
        sl = slice(m * BC, (m + 1) * BC)
        fin_d, cb_d = stage_inputs(cp0[sl], fin[sl], cvec[sl])
        in_maps.append(
            {"fin": fin_d.astype(MM_NP, copy=False), "cb": cb_d,
             "wf": wf, "wf0": wf0, "wh": wh, "wo": wo}
        )

    trace = bool(os.environ.get("DTB_TRACE"))
    res = run_bass_kernel_spmd(
        nc, in_maps, core_ids=list(range(NCORES)), trace=trace
    )
    LAST_RESULTS = res

    outs = [
        unstage_output(np.asarray(res.results[m]["out"], np.float32), BC)
        for m in range(NCORES)
    ]
    return np.concatenate(outs, axis=0)
